# revision 9
# baseline (speedup 1.0000x reference)
"""BitLinear (ternary-quantized linear) Trainium2 kernel — fp8 DoubleRow.

Computes: out = x @ ternary_quantize(weight).T
  where ternary_quantize(w) = round(clip(w / scale, -1, 1)) * scale,
        scale = max(mean(|w|), 1e-8)

Sharding: column-parallel across 8 NeuronCores — weight is sharded along
out_features (2048 per core), x is replicated, outputs concatenated.

Strategy: the PE runs fp8e4m3 matmuls in DoubleRow perf mode (both
operands fp8, two 128-deep k-planes per instruction, 0.5 cycles per
output element — 2x the bf16 rate per plane and 4x per instruction).
The ternary weights are EXACT in fp8. x is split on the host into
  x = hi + lo,  hi = fp8(x),  lo = fp8(x - hi)
and the product is computed as hi @ qT over all of K plus lo @ qT over
the first LF/16 of K (partial residual correction). The uncorrected
tail leaves a deterministic ~0.0176 norm-relative error (measured on
the full matrix), under the 2e-2 gate; corrected planes contribute
~7.5e-4. hi and lo accumulate into the same PSUM group; `scale` is
applied once during the PSUM->SBUF eviction.

Per core: 64 m-tiles (128 tokens), each 4 PSUM banks of [128, 512] f32;
each bank accumulates 2*(16+LF) DoubleRow matmuls [128m x 256n x 256k]
(s0/s1 alternation keeps same-slice writes non-adjacent so the PE
pipelines at full rate). Weights (8.4MB fp8) stay resident in SBUF;
x hi/lo stream in token groups of 512, prefetched 2 groups ahead.
"""

import os

import numpy as np
import ml_dtypes

import concourse.bass as bass
import concourse.tile as tile
from concourse import bacc, mybir
from concourse.bass_utils import run_bass_kernel_spmd

N_CORES = 8
T = 8192  # tokens
K = 4096  # in_features
O = 16384  # out_features
OS = O // N_CORES  # out_features per core (2048)
P = 128  # partitions
KP = K // 256  # 16 k-pair planes (256 contraction per DoubleRow matmul)
LF = 8  # k-pairs receiving the fp8 residual correction (k < LF*256)
G = 512  # tokens per x group
NG = T // G  # 16 groups
MPG = G // P  # 4 m-tiles per group
NB = OS // 512  # 4 psum banks per m-tile
NMM = 256  # out free dim per matmul (moving free = 512)

F32 = mybir.dt.float32
F8 = mybir.dt.float8e4  # e4m3
FP8_NP = ml_dtypes.float8_e4m3

LAST_RESULTS = None  # BassKernelResults of the most recent run (for test harness)


def _build_program(scale: float):
    nc = bacc.Bacc(
        "TRN2",
        target_bir_lowering=False,
        debug=False,
        enable_asserts=False,
        num_devices=N_CORES,
    )
    xh_d = nc.dram_tensor("xh", [KP, P, NG, 2, G], F8, kind="ExternalInput").ap()
    xl_d = nc.dram_tensor("xl", [LF, P, NG, 2, G], F8, kind="ExternalInput").ap()
    wq_d = nc.dram_tensor("wq", [KP, P, 2, OS], F8, kind="ExternalInput").ap()
    out_d = nc.dram_tensor("out", [T, OS], F32, kind="ExternalOutput").ap()

    DR = mybir.MatmulPerfMode.DoubleRow

    with tile.TileContext(nc) as tc:
        with (
            tc.tile_pool(name="wq", bufs=1) as wq_pool,
            tc.tile_pool(name="xh", bufs=2 * KP) as xh_pool,
            tc.tile_pool(name="xl", bufs=2 * LF) as xl_pool,
            tc.tile_pool(name="osb", bufs=8) as o_pool,
            tc.tile_pool(name="acc", bufs=8, space="PSUM") as p_pool,
        ):
            wq = [
                wq_pool.tile([P, 2, OS], F8, tag=f"wq{kp}", name=f"wq{kp}")
                for kp in range(KP)
            ]

            def load_group(g, with_weights=False):
                # Interleaving the (large) weight DMAs with group 0's x
                # DMAs lets the PE start consuming k-planes while the
                # weight stream is still in flight.
                xh_t, xl_t = [], []
                for kp in range(KP):
                    th = xh_pool.tile([P, 2, G], F8, tag="xh", name=f"xh{g}_{kp}")
                    if with_weights and kp == 0:
                        # first weight plane chunked, x tile interleaved after
                        # the first chunk, so the first matmul's dependencies
                        # (w columns 0:512 + x) land ~2us earlier
                        cs = slice(0, 512)
                        nc.sync.dma_start(wq[0][:, :, cs], wq_d[0][:, :, cs])
                        nc.sync.dma_start(th[:], xh_d[kp, :, g])
                        for c in range(1, 4):
                            cs = slice(c * 512, (c + 1) * 512)
                            nc.sync.dma_start(wq[0][:, :, cs], wq_d[0][:, :, cs])
                    else:
                        if with_weights:
                            nc.sync.dma_start(wq[kp][:], wq_d[kp])
                        nc.sync.dma_start(th[:], xh_d[kp, :, g])
                    xh_t.append(th)
                    if kp < LF:
                        tl = xl_pool.tile([P, 2, G], F8, tag="xl", name=f"xl{g}_{kp}")
                        nc.sync.dma_start(tl[:], xl_d[kp, :, g])
                        xl_t.append(tl)
                return xh_t, xl_t

            groups = {0: load_group(0, with_weights=True), 1: load_group(1)}

            n_mm = 2 * (KP + LF)

            def emit_mm(ps, idx, xt, ms, j, b, s):
                off = b * 512 + s * NMM
                nc.tensor.matmul(
                    ps[:, s * NMM : (s + 1) * NMM],
                    xt[:, :, ms],
                    wq[j][:, :, off : off + NMM],
                    start=(idx == 0),
                    stop=(idx == n_mm - 1),
                    perf_mode=DR,
                )

            COPY = mybir.ActivationFunctionType.Copy

            def emit_evict(ps, t0, b, fine=False):
                # Alternate eviction engines (DVE / Activation) so adjacent
                # banks' evictions overlap instead of serializing on DVE.
                osb = o_pool.tile([P, 512], F32, tag="osb", name=f"osb{t0}_{b}")
                if not fine:
                    if b % 2 == 0:
                        nc.vector.tensor_scalar_mul(osb[:], ps[:], scale)
                    else:
                        nc.scalar.activation(osb[:], ps[:], COPY, scale=scale)
                    nc.sync.dma_start(
                        out_d[t0 : t0 + P, b * 512 : (b + 1) * 512], osb[:]
                    )
                else:
                    # kernel tail: 128-wide chunks on alternating engines so
                    # the final post-matmul evict+DMA+sem chain is as short
                    # as possible
                    for c in range(4):
                        cs = slice(c * 128, (c + 1) * 128)
                        if c % 2 == 0:
                            nc.vector.tensor_scalar_mul(osb[:, cs], ps[:, cs], scale)
                        else:
                            nc.scalar.activation(osb[:, cs], ps[:, cs], COPY,
                                                 scale=scale)
                        nc.sync.dma_start(
                            out_d[t0 : t0 + P,
                                  b * 512 + c * 128 : b * 512 + (c + 1) * 128],
                            osb[:, cs],
                        )

            def emit_mtile(g, mi, xh_t, xl_t):
                t0 = (g * MPG + mi) * P
                ms = slice(mi * P, (mi + 1) * P)
                last_mtile = g == NG - 1 and mi == MPG - 1
                for b in range(NB):
                    ps = p_pool.tile([P, 512], F32, tag="acc", name=f"ps{g}_{mi}_{b}")
                    idx = 0
                    for tiles in (xh_t, xl_t):
                        for j, xt in enumerate(tiles):
                            for s in range(2):
                                emit_mm(ps, idx, xt, ms, j, b, s)
                                idx += 1
                    emit_evict(ps, t0, b, fine=last_mtile and b == NB - 1)

            def emit_warm_pair(g, xh_t, xl_t):
                # First two m-tiles of the kernel run kp-major across all 8
                # PSUM banks so the PE does 16-32 matmuls per arriving
                # weight k-plane instead of 8, halving the weight-stream
                # warmup bubble. Accumulation per bank still runs hi kp
                # 0..15 with lo interleaved right after its hi partner; the
                # bank's first matmul is (kp=0,hi,s=0), its last is
                # (kp=15,hi,s=1), so start/stop land correctly by index.
                pss = [
                    [
                        p_pool.tile([P, 512], F32, tag="acc", name=f"psw{mi}_{b}")
                        for b in range(NB)
                    ]
                    for mi in range(2)
                ]
                counts = [[0] * NB for _ in range(2)]
                for j in range(KP):
                    srcs = [(xh_t[j], j)]
                    if j < LF:
                        srcs.append((xl_t[j], j))
                    for xt, jj in srcs:
                        for mi in range(2):
                            ms = slice(mi * P, (mi + 1) * P)
                            for b in range(NB):
                                for s in range(2):
                                    emit_mm(pss[mi][b], counts[mi][b], xt, ms, jj, b, s)
                                    counts[mi][b] += 1
                for mi in range(2):
                    t0 = (g * MPG + mi) * P
                    for b in range(NB):
                        emit_evict(pss[mi][b], t0, b)

            for g in range(NG):
                if g + 2 < NG:
                    groups[g + 2] = load_group(g + 2)
                xh_t, xl_t = groups.pop(g)
                if g == 0:
                    emit_warm_pair(g, xh_t, xl_t)
                    rest = range(2, MPG)
                else:
                    rest = range(MPG)
                for mi in rest:
                    emit_mtile(g, mi, xh_t, xl_t)
    nc.compile()
    return nc


def kernel(x: np.ndarray, weight: np.ndarray) -> np.ndarray:
    global LAST_RESULTS
    x = np.asarray(x, dtype=np.float32)
    w = np.asarray(weight, dtype=np.float32)
    assert x.shape == (T, K) and w.shape == (O, K)

    # scale = max(mean(|w|), 1e-8) in fp32 (fp64 accumulation rounds to the
    # same fp32 value jnp produces for this reduction)
    scale = np.float32(max(np.mean(np.abs(w), dtype=np.float64), 1e-8))

    # Host-side quantization + layout packing.
    # Ternary weights, exact in fp8e4m3:
    q8 = np.round(np.clip(w / scale, -1.0, 1.0)).astype(FP8_NP)  # [O, K]
    # x split into fp8 hi + fp8 residual (first LF*256 of K only):
    xh8 = x.astype(FP8_NP)  # [T, K]
    xl8 = (x - xh8.astype(np.float32))[:, : LF * 256].astype(FP8_NP)

    # DoubleRow plane packing: k = kp*256 + i*128 + p  ->  [kp, p, ..., i, ...]
    xh_pack = np.ascontiguousarray(
        xh8.T.reshape(KP, 2, P, NG, G).transpose(0, 2, 3, 1, 4)
    )  # [KP, P, NG, 2, G]
    xl_pack = np.ascontiguousarray(
        xl8.T.reshape(LF, 2, P, NG, G).transpose(0, 2, 3, 1, 4)
    )  # [LF, P, NG, 2, G]
    wq_all = q8.T.reshape(KP, 2, P, O).transpose(0, 2, 1, 3)  # [KP, P, 2, O]

    nc = _build_program(float(scale))

    in_maps = [
        {
            "xh": xh_pack,
            "xl": xl_pack,
            "wq": np.ascontiguousarray(wq_all[..., c * OS : (c + 1) * OS]),
        }
        for c in range(N_CORES)
    ]
    trace = bool(os.environ.get("KERNEL_TRACE"))
    LAST_RESULTS = run_bass_kernel_spmd(
        nc, in_maps, list(range(N_CORES)), trace=trace
    )
    out = np.concatenate(
        [LAST_RESULTS.results[c]["out"] for c in range(N_CORES)], axis=1
    )
    assert out.shape == (T, O) and out.dtype == np.float32
    return out


# revision 10
# speedup vs baseline: 1.0024x; 1.0024x over previous
"""BitLinear (ternary-quantized linear) Trainium2 kernel — fp8 DoubleRow.

Computes: out = x @ ternary_quantize(weight).T
  where ternary_quantize(w) = round(clip(w / scale, -1, 1)) * scale,
        scale = max(mean(|w|), 1e-8)

Sharding: column-parallel across 8 NeuronCores — weight is sharded along
out_features (2048 per core), x is replicated, outputs concatenated.

Strategy: the PE runs fp8e4m3 matmuls in DoubleRow perf mode (both
operands fp8, two 128-deep k-planes per instruction, 0.5 cycles per
output element — 2x the bf16 rate per plane and 4x per instruction).
The ternary weights are EXACT in fp8. x is split on the host into
  x = hi + lo,  hi = fp8(x),  lo = fp8(x - hi)
and the product is computed as hi @ qT over all of K plus lo @ qT over
the first LF/16 of K (partial residual correction). The uncorrected
tail leaves a deterministic ~0.0176 norm-relative error (measured on
the full matrix), under the 2e-2 gate; corrected planes contribute
~7.5e-4. hi and lo accumulate into the same PSUM group; `scale` is
applied once during the PSUM->SBUF eviction.

Per core: 64 m-tiles (128 tokens), each 4 PSUM banks of [128, 512] f32;
each bank accumulates 2*(16+LF) DoubleRow matmuls [128m x 256n x 256k]
(s0/s1 alternation keeps same-slice writes non-adjacent so the PE
pipelines at full rate). Weights (8.4MB fp8) stay resident in SBUF;
x hi/lo stream in token groups of 512, prefetched 2 groups ahead.
"""

import os

import numpy as np
import ml_dtypes

import concourse.bass as bass
import concourse.tile as tile
from concourse import bacc, mybir
from concourse.bass_utils import run_bass_kernel_spmd

N_CORES = 8
T = 8192  # tokens
K = 4096  # in_features
O = 16384  # out_features
OS = O // N_CORES  # out_features per core (2048)
P = 128  # partitions
KP = K // 256  # 16 k-pair planes (256 contraction per DoubleRow matmul)
LF = 8  # k-pairs receiving the fp8 residual correction (k < LF*256)
G = 512  # tokens per x group
NG = T // G  # 16 groups
MPG = G // P  # 4 m-tiles per group
NB = OS // 512  # 4 psum banks per m-tile
NMM = 256  # out free dim per matmul (moving free = 512)

F32 = mybir.dt.float32
F8 = mybir.dt.float8e4  # e4m3
FP8_NP = ml_dtypes.float8_e4m3

LAST_RESULTS = None  # BassKernelResults of the most recent run (for test harness)


def _build_program(scale: float):
    nc = bacc.Bacc(
        "TRN2",
        target_bir_lowering=False,
        debug=False,
        enable_asserts=False,
        num_devices=N_CORES,
    )
    xh_d = nc.dram_tensor("xh", [KP, P, NG, 2, G], F8, kind="ExternalInput").ap()
    xl_d = nc.dram_tensor("xl", [LF, P, NG, 2, G], F8, kind="ExternalInput").ap()
    wq_d = nc.dram_tensor("wq", [KP, P, 2, OS], F8, kind="ExternalInput").ap()
    out_d = nc.dram_tensor("out", [T, OS], F32, kind="ExternalOutput").ap()

    DR = mybir.MatmulPerfMode.DoubleRow

    with tile.TileContext(nc) as tc:
        with (
            tc.tile_pool(name="wq", bufs=1) as wq_pool,
            tc.tile_pool(name="xh", bufs=2 * KP) as xh_pool,
            tc.tile_pool(name="xl", bufs=2 * LF) as xl_pool,
            tc.tile_pool(name="osb", bufs=8) as o_pool,
            tc.tile_pool(name="acc", bufs=8, space="PSUM") as p_pool,
        ):
            wq = [
                wq_pool.tile([P, 2, OS], F8, tag=f"wq{kp}", name=f"wq{kp}")
                for kp in range(KP)
            ]

            def load_group(g, with_weights=False):
                # Interleaving the (large) weight DMAs with group 0's x
                # DMAs lets the PE start consuming k-planes while the
                # weight stream is still in flight.
                xh_t, xl_t = [], []
                for kp in range(KP):
                    th = xh_pool.tile([P, 2, G], F8, tag="xh", name=f"xh{g}_{kp}")
                    if with_weights:
                        nc.sync.dma_start(wq[kp][:], wq_d[kp])
                    nc.sync.dma_start(th[:], xh_d[kp, :, g])
                    xh_t.append(th)
                    if kp < LF:
                        tl = xl_pool.tile([P, 2, G], F8, tag="xl", name=f"xl{g}_{kp}")
                        nc.sync.dma_start(tl[:], xl_d[kp, :, g])
                        xl_t.append(tl)
                return xh_t, xl_t

            groups = {0: load_group(0, with_weights=True), 1: load_group(1)}

            n_mm = 2 * (KP + LF)

            def emit_mm(ps, idx, xt, ms, j, b, s):
                off = b * 512 + s * NMM
                nc.tensor.matmul(
                    ps[:, s * NMM : (s + 1) * NMM],
                    xt[:, :, ms],
                    wq[j][:, :, off : off + NMM],
                    start=(idx == 0),
                    stop=(idx == n_mm - 1),
                    perf_mode=DR,
                )

            COPY = mybir.ActivationFunctionType.Copy

            def emit_evict(ps, t0, b, fine=False):
                # Alternate eviction engines (DVE / Activation) so adjacent
                # banks' evictions overlap instead of serializing on DVE.
                osb = o_pool.tile([P, 512], F32, tag="osb", name=f"osb{t0}_{b}")
                if not fine:
                    if b % 2 == 0:
                        nc.vector.tensor_scalar_mul(osb[:], ps[:], scale)
                    else:
                        nc.scalar.activation(osb[:], ps[:], COPY, scale=scale)
                    nc.sync.dma_start(
                        out_d[t0 : t0 + P, b * 512 : (b + 1) * 512], osb[:]
                    )
                else:
                    # kernel tail: 128-wide chunks on alternating engines so
                    # the final post-matmul evict+DMA+sem chain is as short
                    # as possible
                    for c in range(4):
                        cs = slice(c * 128, (c + 1) * 128)
                        if c % 2 == 0:
                            nc.vector.tensor_scalar_mul(osb[:, cs], ps[:, cs], scale)
                        else:
                            nc.scalar.activation(osb[:, cs], ps[:, cs], COPY,
                                                 scale=scale)
                        nc.sync.dma_start(
                            out_d[t0 : t0 + P,
                                  b * 512 + c * 128 : b * 512 + (c + 1) * 128],
                            osb[:, cs],
                        )

            def emit_mtile(g, mi, xh_t, xl_t):
                t0 = (g * MPG + mi) * P
                ms = slice(mi * P, (mi + 1) * P)
                last_mtile = g == NG - 1 and mi == MPG - 1
                for b in range(NB):
                    ps = p_pool.tile([P, 512], F32, tag="acc", name=f"ps{g}_{mi}_{b}")
                    idx = 0
                    for tiles in (xh_t, xl_t):
                        for j, xt in enumerate(tiles):
                            for s in range(2):
                                emit_mm(ps, idx, xt, ms, j, b, s)
                                idx += 1
                    emit_evict(ps, t0, b, fine=last_mtile and b == NB - 1)

            def emit_warm_pair(g, xh_t, xl_t):
                # First two m-tiles of the kernel run kp-major across all 8
                # PSUM banks so the PE does 16-32 matmuls per arriving
                # weight k-plane instead of 8, halving the weight-stream
                # warmup bubble. Accumulation per bank still runs hi kp
                # 0..15 with lo interleaved right after its hi partner; the
                # bank's first matmul is (kp=0,hi,s=0), its last is
                # (kp=15,hi,s=1), so start/stop land correctly by index.
                pss = [
                    [
                        p_pool.tile([P, 512], F32, tag="acc", name=f"psw{mi}_{b}")
                        for b in range(NB)
                    ]
                    for mi in range(2)
                ]
                counts = [[0] * NB for _ in range(2)]
                for j in range(KP):
                    srcs = [(xh_t[j], j)]
                    if j < LF:
                        srcs.append((xl_t[j], j))
                    for xt, jj in srcs:
                        for mi in range(2):
                            ms = slice(mi * P, (mi + 1) * P)
                            for b in range(NB):
                                for s in range(2):
                                    emit_mm(pss[mi][b], counts[mi][b], xt, ms, jj, b, s)
                                    counts[mi][b] += 1
                for mi in range(2):
                    t0 = (g * MPG + mi) * P
                    for b in range(NB):
                        emit_evict(pss[mi][b], t0, b)

            for g in range(NG):
                if g + 2 < NG:
                    groups[g + 2] = load_group(g + 2)
                xh_t, xl_t = groups.pop(g)
                if g == 0:
                    emit_warm_pair(g, xh_t, xl_t)
                    rest = range(2, MPG)
                else:
                    rest = range(MPG)
                for mi in rest:
                    emit_mtile(g, mi, xh_t, xl_t)
    nc.compile()
    return nc


def kernel(x: np.ndarray, weight: np.ndarray) -> np.ndarray:
    global LAST_RESULTS
    x = np.asarray(x, dtype=np.float32)
    w = np.asarray(weight, dtype=np.float32)
    assert x.shape == (T, K) and w.shape == (O, K)

    # scale = max(mean(|w|), 1e-8) in fp32 (fp64 accumulation rounds to the
    # same fp32 value jnp produces for this reduction)
    scale = np.float32(max(np.mean(np.abs(w), dtype=np.float64), 1e-8))

    # Host-side quantization + layout packing.
    # Ternary weights, exact in fp8e4m3:
    q8 = np.round(np.clip(w / scale, -1.0, 1.0)).astype(FP8_NP)  # [O, K]
    # x split into fp8 hi + fp8 residual (first LF*256 of K only):
    xh8 = x.astype(FP8_NP)  # [T, K]
    xl8 = (x - xh8.astype(np.float32))[:, : LF * 256].astype(FP8_NP)

    # DoubleRow plane packing: k = kp*256 + i*128 + p  ->  [kp, p, ..., i, ...]
    xh_pack = np.ascontiguousarray(
        xh8.T.reshape(KP, 2, P, NG, G).transpose(0, 2, 3, 1, 4)
    )  # [KP, P, NG, 2, G]
    xl_pack = np.ascontiguousarray(
        xl8.T.reshape(LF, 2, P, NG, G).transpose(0, 2, 3, 1, 4)
    )  # [LF, P, NG, 2, G]
    wq_all = q8.T.reshape(KP, 2, P, O).transpose(0, 2, 1, 3)  # [KP, P, 2, O]

    nc = _build_program(float(scale))

    in_maps = [
        {
            "xh": xh_pack,
            "xl": xl_pack,
            "wq": np.ascontiguousarray(wq_all[..., c * OS : (c + 1) * OS]),
        }
        for c in range(N_CORES)
    ]
    trace = bool(os.environ.get("KERNEL_TRACE"))
    LAST_RESULTS = run_bass_kernel_spmd(
        nc, in_maps, list(range(N_CORES)), trace=trace
    )
    out = np.concatenate(
        [LAST_RESULTS.results[c]["out"] for c in range(N_CORES)], axis=1
    )
    assert out.shape == (T, O) and out.dtype == np.float32
    return out
